# revision 1
# baseline (speedup 1.0000x reference)
"""AdaptiveLIF spiking-neuron kernel for 8 TRN2 NeuronCores.

Reference recurrence (per element, over T steps):
    v = v*decay + I_t ; s = (v - vth > 0) ; v = v*(1-s)

Sharding: data-parallel over B (B=8 -> 1 batch element per core). The
recurrence is only over T, so no cross-core communication.

Per-core layout: (C,H,W) = (64,64,64) flattened to (128 partitions, 2048),
partition p holds channel c = p//2, so decay/vth are per-partition scalars.

Per step on each core (two half-width column chunks so the serial
T-recurrence pipelines across DVE and ACT):
  DVE : v  = (w * decay) + x          (scalar_tensor_tensor, fused)
  ACT : g  = Sign(v - vth)            (activation, bias = -vth)
  ACT : s  = Relu(g)                  (exactly 0.0/1.0 spike output, bf16)
  DVE : w  = (g <= 0) * v             (scalar_tensor_tensor -> reset)

DMA plan (the kernel is fabric/HBM-bound at ~2 MiB per 4.9 us step):
  - input loads on SyncE's HWDGE ring, prefetched several steps ahead;
  - mid-kernel spike stores on GpSimd (SWDGE) with a bf16->f32 cast in
    the DMA — a separate issue path (input prefetch never stalls behind
    a store) and half the SBUF-fabric bytes on the store side;
  - the last two steps store f32 via ScalarE's HWDGE ring, which
    completes much faster than SWDGE and shortens the kernel tail.
All spike values are exactly 0.0/1.0, so the bf16 staging is lossless.
"""

import numpy as np
from contextlib import ExitStack

import concourse.bass as bass
import concourse.tile as tile
from concourse import bacc, mybir
from concourse.bass_utils import run_bass_kernel_spmd

T, B, C, H, W = 16, 8, 64, 64, 64
P = 128                 # SBUF partitions
FD = (C * H * W) // P   # free dim per step per core = 2048
NCHUNK = 2              # column chunks for cross-engine pipelining
CH = FD // NCHUNK
N_CORES = 8

_nc_cache = None


def _build_nc(x_bufs=12, s_bufs=6, mid_store="gpsimd", ramp_split=False):
    nc = bacc.Bacc("TRN2", target_bir_lowering=False, debug=False)
    f32 = mybir.dt.float32
    I_ext = nc.dram_tensor("I", [T, P, FD], f32, kind="ExternalInput").ap()
    decay_ext = nc.dram_tensor("decay", [P, 1], f32, kind="ExternalInput").ap()
    nvth_ext = nc.dram_tensor("nvth", [P, 1], f32, kind="ExternalInput").ap()
    out_ext = nc.dram_tensor("out", [T, P, FD], f32, kind="ExternalOutput").ap()

    with tile.TileContext(nc) as tc, ExitStack() as ctx:
        const_pool = ctx.enter_context(tc.tile_pool(name="const", bufs=1))
        state_pool = ctx.enter_context(tc.tile_pool(name="state", bufs=1))
        x_pool = ctx.enter_context(tc.tile_pool(name="x", bufs=x_bufs))
        s_pool = ctx.enter_context(tc.tile_pool(name="s", bufs=s_bufs))
        v_pool = ctx.enter_context(tc.tile_pool(name="v", bufs=3))
        g_pool = ctx.enter_context(tc.tile_pool(name="g", bufs=3))

        # When ramp_split is on, the very first input bytes are fetched as
        # two quarter-width transfers issued before anything else, so the
        # first recurrence block fires ~2 us sooner (the whole ramp is
        # serial on this one load).
        xq = []
        if ramp_split:
            for q in range(2):
                xt = x_pool.tile([P, CH // 2], f32, tag="x0")
                nc.sync.dma_start(
                    out=xt[:], in_=I_ext[0][:, q * (CH // 2):(q + 1) * (CH // 2)])
                xq.append(xt)

        decay_sb = const_pool.tile([P, 1], f32, tag="decay")
        nvth_sb = const_pool.tile([P, 1], f32, tag="nvth")
        nc.sync.dma_start(out=decay_sb[:], in_=decay_ext[:])
        nc.sync.dma_start(out=nvth_sb[:], in_=nvth_ext[:])

        ws = []
        for chk in range(NCHUNK):
            wt = state_pool.tile([P, CH], f32, tag=f"w{chk}")
            nc.vector.memset(wt[:], 0.0)
            ws.append(wt)

        def lif_block(t, chk, width, off, x, tag=""):
            """One recurrence block over columns [off, off+width) of chunk chk.
            x is the input tile for exactly those columns."""
            sl = slice(off, off + width)
            if t == 0:
                # v_0 = 0*decay + x_0 == x_0 bit-exactly: skip the STT so the
                # ramp-critical first block starts on ACT as soon as x lands.
                v_ap = x
            else:
                v = v_pool.tile([P, width], f32, tag="v" + tag)
                nc.vector.scalar_tensor_tensor(
                    v[:], ws[chk][:, sl], decay_sb[:], x[:],
                    op0=mybir.AluOpType.mult, op1=mybir.AluOpType.add,
                )
                v_ap = v
            g = g_pool.tile([P, width], f32, tag="g" + tag)
            nc.scalar.activation(
                g[:], v_ap[:], mybir.ActivationFunctionType.Sign,
                bias=nvth_sb[:],
            )
            # NOTE: offloading part of this STT to GpSimd is NOT possible:
            # TensorScalarPtr is not a legal Pool-engine opcode on NC-v3
            # (walrus NCC_IXCG966), even though bass exposes the method.
            nc.vector.scalar_tensor_tensor(
                ws[chk][:, sl], g[:], 0.0, v_ap[:],
                op0=mybir.AluOpType.is_le, op1=mybir.AluOpType.mult,
            )
            return g

        bf16 = mybir.dt.bfloat16
        for t in range(T):
            # Per-chunk input tiles: the chunk-0 recurrence starts as soon as
            # its half arrives instead of waiting for the full 1 MiB step.
            nsub = 1
            sw = CH // nsub
            # Spikes are exactly 0.0/1.0, so bf16 holds them losslessly.
            # Mid-kernel stores go out via SWDGE with a bf16->f32 cast in the
            # DMA, halving the SBUF-fabric bytes on the store side (the
            # fabric, not HBM, is the binding constraint at this cadence).
            # The last two steps store f32 via HWDGE, which cannot cast.
            s_dt = bf16 if t < T - 2 else f32
            s = s_pool.tile([P, FD], s_dt, tag="s")
            for chk in range(NCHUNK):
                if ramp_split and t == 0 and chk == 0:
                    # Quarter-width blocks on the pre-issued transfers.
                    qw = CH // 2
                    for q in range(2):
                        g = lif_block(t, chk, qw, q * qw, xq[q], tag="0")
                        nc.scalar.activation(
                            s[:, q * qw:(q + 1) * qw], g[:],
                            mybir.ActivationFunctionType.Relu,
                        )
                    continue
                for sub in range(nsub):
                    x = x_pool.tile([P, sw], f32, tag="x")
                    base = chk * CH + sub * sw
                    nc.sync.dma_start(out=x[:], in_=I_ext[t][:, base:base + sw])
                    g = lif_block(t, chk, sw, sub * sw, x)
                    nc.scalar.activation(
                        s[:, base:base + sw], g[:],
                        mybir.ActivationFunctionType.Relu,
                    )
                    if t >= T - 2:
                        # Tail steps: store each half right after its Relu on
                        # ScalarE's HWDGE ring, so the final transfer is only
                        # 512 KiB and starts as early as possible.
                        nc.scalar.dma_start(
                            out=out_ext[t][:, base:base + sw],
                            in_=s[:, base:base + sw],
                        )
            # One contiguous 1 MiB store per step mid-kernel, via GpSimd
            # (SWDGE): a separate issue path, so the SyncE FIFO keeps
            # streaming input prefetch at full rate, and the DMA casts the
            # bf16 spikes back to f32 on the way out.
            if t < T - 2:
                getattr(nc, mid_store).dma_start(out=out_ext[t], in_=s[:])

    nc.compile()
    return nc


def get_nc():
    global _nc_cache
    if _nc_cache is None:
        _nc_cache = _build_nc()
    return _nc_cache


def _prep_in_maps(I, tau, vth):
    I = np.ascontiguousarray(np.asarray(I, dtype=np.float32))
    tau = np.asarray(tau, dtype=np.float32)
    vth = np.asarray(vth, dtype=np.float32)
    # Match the reference's broadcast + clamp, in fp32:
    tau_bc = np.broadcast_to(tau, (B, C)) if tau.shape[1] == 1 else tau
    vth_bc = np.broadcast_to(vth, (B, C)) if vth.shape[1] == 1 else vth
    tau_bc = np.maximum(tau_bc, np.float32(0.001))
    vth_bc = np.maximum(vth_bc, np.float32(0.001))
    decay = np.exp(np.float32(-1.0) / tau_bc).astype(np.float32)   # (B, C)

    in_maps = []
    for b in range(B):
        in_maps.append({
            "I": np.ascontiguousarray(I[:, b]).reshape(T, P, FD),
            "decay": np.repeat(decay[b], P // C).reshape(P, 1).astype(np.float32),
            "nvth": np.repeat(-vth_bc[b], P // C).reshape(P, 1).astype(np.float32),
        })
    return in_maps


def run(I, tau, vth, **spmd_kwargs):
    nc = get_nc()
    in_maps = _prep_in_maps(I, tau, vth)
    res = run_bass_kernel_spmd(nc, in_maps, core_ids=list(range(N_CORES)),
                               **spmd_kwargs)
    out = np.stack(
        [res.results[b]["out"].reshape(T, C, H, W) for b in range(B)], axis=1
    ).astype(np.float32)
    return out, res


def kernel(I, tau, vth):
    out, _ = run(I, tau, vth)
    return out

